# revision 11
# baseline (speedup 1.0000x reference)
"""Equilibrium Propagation value network — Trainium2 Bass kernel.

Data-parallel over 8 NeuronCores: core c owns batch rows [2048c, 2048c+2048).
All state lives in SBUF in transposed [feature, batch] layout so the two
per-iteration matmuls need no transposes:

    pre_h^T[h, b] = (rho(x)@W1 + b_h)^T + W2T.T @ rho(o)^T     (C^T cached)
    pre_o^T[o, b] = W2.T-tiles @ rho(h)^T + b_o

Per iteration (15x): one fused custom-DVE op computes the energy gradient
  g = h - [clip01(h)==h] * (pre + b), Adam moments update in place
(scaled m/(1-B1), v/(1-B2) so updates are single fused ops), and the Adam
step uses ACT sqrt + a 51-ULP DVE reciprocal.  t=1 and t=2 are specialised:
h,o are exactly zero there, where JAX's clip subgradient is 0.5, and the
states are first *written* in those steps so no zero-init memsets are needed.

fp32 end to end: the dynamics are chaotic at ~3e-4 rel under any fp32
reordering (clip-mask flips), and reduced-precision matmuls would amplify
that past ~1e-2, so matmuls stay exact fp32.
"""

import re

import numpy as np

import concourse.bass as bass
import concourse.mybir as mybir
from concourse import bacc
from concourse import bass_utils
from concourse.bass import ds
from concourse.dve_ops import OPS, DveOp
from concourse.dve_spec import C0, C1, One, Spec, Src0, Src1, Zero, eq, maxx, minn, sq
from concourse.tile import TileContext

F32 = mybir.dt.float32
AF = mybir.ActivationFunctionType
OP = mybir.AluOpType

B1, B2, EPS, LR = 0.9, 0.999, 1e-8, 0.01
BATCH, INPUT, HIDDEN, OUT = 16384, 256, 512, 128
NCORES = 8
SH = BATCH // NCORES  # 2048 batch rows per core
NCHUNK = 512          # matmul moving-dim chunk (one PSUM bank, fp32 max)


# --------------------------------------------------------------------------- #
# Custom DVE ops
# --------------------------------------------------------------------------- #
def _register_op(name: str, spec: Spec, subdim: bool = False) -> DveOp:
    import concourse.dve_ops as dve_ops

    for existing in OPS:
        if existing.name == name:
            return existing
    probe = DveOp(name, spec, subdim, uops_sha={})
    OPS.append(probe)
    dve_ops.CUSTOM_DVE_SPECS[name] = spec
    dve_ops._SUB_OPCODE_FOR_NAME[name] = dve_ops._CUSTOM_DVE_ROW_BASE + len(OPS) - 1
    shas = {}
    for ver in ("v3", "v4"):
        try:
            probe.compile(ver)
        except ValueError as e:
            m = re.search(r'="([0-9a-f]+)"', str(e))
            if m:
                shas[ver] = m.group(1)
    final = DveOp(name, spec, subdim, uops_sha=shas)
    OPS[-1] = final
    return final


# g = h - [clip01(h) == h] * (pre + bias);   in0=h, in1=pre, s0=bias
_clip_h = minn(maxx(Src0, Zero), One)
EQPROP_G = _register_op(
    "EQPROP_G",
    Spec(
        body=Src0 - (eq(_clip_h, Src0) * (Src1 + C0)),
        reference=lambda in0, in1, s0, s1, imm2: in0
        - (np.minimum(np.maximum(in0, 0), 1) == in0) * (in1 + s0),
    ),
)

# g = (in0 + s0) * s1  — used where the state is exactly 0 (t=1 h, t=2 o)
# with s1 = -0.5 (JAX clip subgradient at 0 is 0.5).
EQPROP_G0 = _register_op(
    "EQPROP_G0",
    Spec(
        body=(Src0 + C0) * C1,
        reference=lambda in0, in1, s0, s1, imm2: (in0 + s0) * s1,
    ),
)

# V' = V*s0 + g^2  (scaled second moment update);  in0=V, in1=g
EQPROP_V = _register_op(
    "EQPROP_V",
    Spec(
        body=Src0 * C0 + sq(Src1),
        reference=lambda in0, in1, s0, s1, imm2: in0 * s0 + in1 * in1,
    ),
)


# --------------------------------------------------------------------------- #
# Bass program (identical on every core; per-core data via in_maps)
# --------------------------------------------------------------------------- #
def _build(num_iter: int) -> bass.Bass:
    nc = bacc.Bacc()

    xT = nc.dram_tensor("xT", [INPUT, SH], F32, kind="ExternalInput")
    w1 = nc.dram_tensor("w1", [128, 2 * HIDDEN], F32, kind="ExternalInput")
    w2 = nc.dram_tensor("w2", [128, 4 * OUT], F32, kind="ExternalInput")
    w2t = nc.dram_tensor("w2t", [OUT, HIDDEN], F32, kind="ExternalInput")
    bh_d = nc.dram_tensor("bh", [128, 4], F32, kind="ExternalInput")
    bo_d = nc.dram_tensor("bo", [128, 1], F32, kind="ExternalInput")
    id_d = nc.dram_tensor("iden", [128, 128], F32, kind="ExternalInput")
    oout = nc.dram_tensor("oout", [SH, OUT], F32, kind="ExternalOutput")

    with TileContext(nc) as tc:
        with (
            tc.tile_pool(name="state", bufs=1) as st,
            tc.tile_pool(name="rhp", bufs=2) as rhp,
            tc.tile_pool(name="rop", bufs=1) as rop,
            tc.tile_pool(name="gsp", bufs=3) as gsp,
            tc.tile_pool(name="psum", bufs=1, space="PSUM") as pp,
        ):
            hT = [st.tile([128, SH], F32, tag=f"h{j}", name=f"h{j}") for j in range(4)]
            MT = [st.tile([128, SH], F32, tag=f"M{j}", name=f"M{j}") for j in range(4)]
            VT = [st.tile([128, SH], F32, tag=f"V{j}", name=f"V{j}") for j in range(4)]
            CT = [st.tile([128, SH], F32, tag=f"C{j}", name=f"C{j}") for j in range(4)]
            oT = st.tile([128, SH], F32, tag="o", name="oT")
            MoT = st.tile([128, SH], F32, tag="Mo", name="MoT")
            VoT = st.tile([128, SH], F32, tag="Vo", name="VoT")
            w2sb = st.tile([128, 4 * OUT], F32, tag="w2sb", name="w2sb")
            w2tsb = st.tile([OUT, HIDDEN], F32, tag="w2tsb", name="w2tsb")
            bhsb = st.tile([128, 4], F32, tag="bh", name="bhsb")
            bosb = st.tile([128, 1], F32, tag="bo", name="bosb")
            idsb = st.tile([128, 128], F32, tag="iden", name="idsb")

            nc.sync.dma_start(w2sb[:], w2[:, :])
            nc.sync.dma_start(w2tsb[:], w2t[:, :])
            nc.sync.dma_start(bhsb[:], bh_d[:, :])
            nc.sync.dma_start(bosb[:], bo_d[:, :])
            nc.sync.dma_start(idsb[:], id_d[:, :])

            # Touch each PE-read tensor once, with a single dependency per
            # matmul: the LDWEIGHTS struct encodes only ONE sync wait, so loop
            # matmuls must never need waits on more than one semaphore.
            ptw = pp.tile([128, 8], F32, tag="po", name="ptw")
            nc.tensor.matmul(ptw[:, 0:1], idsb[:], idsb[:, 0:1], start=True, stop=True)
            nc.tensor.matmul(
                ptw[:, 1:2], w2tsb[:, ds(0, 128)], w2tsb[:, 0:1], start=True, stop=True
            )
            nc.tensor.matmul(
                ptw[:, 2:3], w2sb[:, ds(0, 128)], w2sb[:, 0:1], start=True, stop=True
            )

            # ---- prologue: C^T = (clip01(x) @ W1)^T, per 128-feature tile ----
            rx = [rhp.tile([128, SH], F32, tag="rh", name="rx") for _ in range(2)]
            nc.sync.dma_start(rx[0][:], xT[0:128, :])
            nc.sync.dma_start(rx[1][:], xT[128:256, :])
            w1sb = gsp.tile([128, 2 * HIDDEN], F32, tag="gs", name="w1sb")
            nc.sync.dma_start(w1sb[:], w1[:, :])
            nc.tensor.matmul(
                ptw[:, 3:4], w1sb[:, ds(0, 128)], w1sb[:, 0:1], start=True, stop=True
            )
            nc.vector.tensor_scalar(rx[0][:], rx[0][:], 0.0, 1.0, op0=OP.max, op1=OP.min)
            nc.vector.tensor_scalar(rx[1][:], rx[1][:], 0.0, 1.0, op0=OP.max, op1=OP.min)
            for j in range(4):
                pc = pp.tile([128, SH], F32, tag="ph", name="ph")
                for n in range(SH // NCHUNK):
                    sl = ds(n * NCHUNK, NCHUNK)
                    nc.tensor.matmul(
                        pc[:, sl], w1sb[:, ds(j * 128, 128)], rx[0][:, sl],
                        start=True, stop=False,
                    )
                    nc.tensor.matmul(
                        pc[:, sl], w1sb[:, ds(HIDDEN + j * 128, 128)], rx[1][:, sl],
                        start=False, stop=True,
                    )
                nc.scalar.copy(CT[j][:], pc[:])

            # ---- the equilibrium-propagation / Adam loop ----
            for t in range(1, num_iter + 1):
                a_t = LR * (1.0 - B1) / (1.0 - B1**t)   # folds m-hat scaling
                g_t = (1.0 - B2) / (1.0 - B2**t)        # folds v-hat scaling
                sqrt_scale = g_t / (a_t * a_t)
                epsn = EPS / a_t

                def adam_tail(M, V, target, first_write: bool, j_dbg: int,
                              sub_on_dve: bool = False):
                    """sqrt/recip/update chain shared by every state tile.

                    first_write: target is written as  M * (-r)  (state was 0),
                    otherwise target -= M * r, in place.
                    """
                    s = gsp.tile([128, SH], F32, tag="gs", name="sbuf_s")
                    nc.scalar.activation(s[:], V[:], AF.Sqrt, scale=sqrt_scale)
                    r = gsp.tile([128, SH], F32, tag="gs", name="sbuf_r")
                    if first_write:
                        nc.scalar.activation(r[:], s[:], AF.Copy, bias=-epsn, scale=-1.0)
                    else:
                        nc.scalar.activation(r[:], s[:], AF.Copy, bias=epsn, scale=1.0)
                    nc.vector.reciprocal_approx_fast(out=r[:], in_=r[:])
                    if first_write:
                        nc.gpsimd.tensor_mul(target[:], M[:], r[:])
                    else:
                        nc.gpsimd.tensor_mul(r[:], M[:], r[:])
                        if sub_on_dve:
                            nc.vector.tensor_sub(target[:], target[:], r[:])
                        else:
                            nc.gpsimd.tensor_sub(target[:], target[:], r[:])

                if t == 1:
                    # h = o = 0; g_h = -0.5*(C + b_h); o untouched (stays 0).
                    for j in range(4):
                        nc.vector._custom_dve(
                            EQPROP_G0, out=MT[j][:], in0=CT[j][:],
                            s0=bhsb[:, ds(j, 1)], s1=-0.5,
                        )
                        nc.scalar.activation(VT[j][:], MT[j][:], AF.Square)
                        adam_tail(MT[j], VT[j], hT[j], True, j)
                    continue

                # rho(h) tiles (also needed at t=2); rho(o) only when o != 0
                rh = []
                for j in range(4):
                    rhj = rhp.tile([128, SH], F32, tag="rh", name="rh")
                    nc.vector.tensor_scalar(
                        rhj[:], hT[j][:], 0.0, 1.0, op0=OP.max, op1=OP.min
                    )
                    rh.append(rhj)
                if t > 2:
                    ro = rop.tile([128, SH], F32, tag="ro", name="ro")
                    nc.gpsimd.tensor_scalar(
                        ro[:], oT[:], 0.0, 1.0, op0=OP.max, op1=OP.min
                    )

                # pre_o^T = W2-tiles.T @ rho(h)^T   (K = 512, accumulated)
                po = pp.tile([128, SH], F32, tag="po", name="po")
                for j in range(4):
                    for n in range(SH // NCHUNK):
                        sl = ds(n * NCHUNK, NCHUNK)
                        nc.tensor.matmul(
                            po[:, sl], w2sb[:, ds(j * 128, 128)], rh[j][:, sl],
                            start=(j == 0), stop=(j == 3),
                        )

                # o-chain
                if t == 2:
                    nc.vector._custom_dve(
                        EQPROP_G0, out=MoT[:], in0=po[:], s0=bosb[:, ds(0, 1)], s1=-0.5
                    )
                    nc.scalar.activation(VoT[:], MoT[:], AF.Square)
                    adam_tail(MoT, VoT, oT, True, -1)
                else:
                    go = gsp.tile([128, SH], F32, tag="gs", name="go")
                    nc.vector._custom_dve(
                        EQPROP_G, out=go[:], in0=oT[:], in1=po[:], s0=bosb[:, ds(0, 1)]
                    )
                    nc.vector.scalar_tensor_tensor(
                        MoT[:], MoT[:], B1, go[:], op0=OP.mult, op1=OP.add
                    )
                    nc.vector._custom_dve(
                        EQPROP_V, out=VoT[:], in0=VoT[:], in1=go[:], s0=B2
                    )
                    adam_tail(MoT, VoT, oT, False, -1, sub_on_dve=(t == num_iter))

                # h-chains (pre_h^T = C^T [+ W2T.T @ rho(o)^T when o != 0])
                for j in range(4):
                    if t > 2:
                        ph = pp.tile([128, SH], F32, tag="ph", name="ph")
                        for n in range(SH // NCHUNK):
                            sl = ds(n * NCHUNK, NCHUNK)
                            nc.tensor.matmul(
                                ph[:, sl], idsb[:], CT[j][:, sl],
                                start=True, stop=False,
                            )
                            nc.tensor.matmul(
                                ph[:, sl], w2tsb[:, ds(j * 128, 128)], ro[:, sl],
                                start=False, stop=True,
                            )
                        pre = ph
                    else:
                        pre = CT[j]
                    g = gsp.tile([128, SH], F32, tag="gs", name="gh")
                    nc.vector._custom_dve(
                        EQPROP_G, out=g[:], in0=hT[j][:], in1=pre[:],
                        s0=bhsb[:, ds(j, 1)],
                    )
                    nc.vector.scalar_tensor_tensor(
                        MT[j][:], MT[j][:], B1, g[:], op0=OP.mult, op1=OP.add
                    )
                    nc.vector._custom_dve(
                        EQPROP_V, out=VT[j][:], in0=VT[j][:], in1=g[:], s0=B2
                    )
                    adam_tail(MT[j], VT[j], hT[j], False, j)

            # ---- epilogue: transpose o^T -> [batch, 128] and DMA out ----
            oo = gsp.tile([128, SH], F32, tag="gs", name="oo")
            for b in range(SH // 128):
                pt = pp.tile([128, 128], F32, tag="po", name="pt")
                nc.tensor.transpose(pt[:], oT[:, ds(b * 128, 128)], idsb[:])
                nc.vector.tensor_copy(oo[:, ds(b * 128, 128)], pt[:])
            oo3 = oo[:].rearrange("p (t o) -> p t o", o=OUT)
            od3 = oout.rearrange("(t p) o -> p t o", p=128)
            nc.sync.dma_start(od3, oo3)

    nc.finalize()
    return nc


_CACHE: dict[int, bass.Bass] = {}


def kernel(**inputs: np.ndarray) -> np.ndarray:
    out, _ = _run(inputs, trace=False)
    return out


def _make_in_maps(inputs):
    x = np.asarray(inputs["x"], dtype=np.float32)
    W1 = np.asarray(inputs["W1"], dtype=np.float32)
    W2 = np.asarray(inputs["W2"], dtype=np.float32)
    b_h = np.asarray(inputs["b_h"], dtype=np.float32)
    b_o = np.asarray(inputs["b_o"], dtype=np.float32)
    w1sb = np.ascontiguousarray(
        W1.reshape(2, 128, HIDDEN).transpose(1, 0, 2).reshape(128, 2 * HIDDEN)
    )
    w2sb = np.ascontiguousarray(
        W2.reshape(4, 128, OUT).transpose(1, 0, 2).reshape(128, 4 * OUT)
    )
    w2t = np.ascontiguousarray(W2.T)
    bh = np.ascontiguousarray(b_h.reshape(4, 128).T)
    bo = np.ascontiguousarray(b_o.reshape(128, 1))
    iden = np.eye(128, dtype=np.float32)

    in_maps = []
    for c in range(NCORES):
        in_maps.append(
            {
                "xT": np.ascontiguousarray(x[c * SH : (c + 1) * SH].T),
                "w1": w1sb,
                "w2": w2sb,
                "w2t": w2t,
                "bh": bh,
                "bo": bo,
                "iden": iden,
            }
        )
    return in_maps


def _run(inputs, trace: bool = False):
    num_iter = int(np.asarray(inputs["num_iterations"]))
    nc = _CACHE.get(num_iter)
    if nc is None:
        nc = _build(num_iter)
        _CACHE[num_iter] = nc
    in_maps = _make_in_maps(inputs)
    res = bass_utils.run_bass_kernel_spmd(
        nc, in_maps, core_ids=list(range(NCORES)), trace=trace
    )
    out = np.concatenate([res.results[c]["oout"] for c in range(NCORES)], axis=0)
    return out, res
